# revision 30
# baseline (speedup 1.0000x reference)
"""Trainium2 Bass kernel v6: dual-layout x load, all-PE datapath.

Both x layouts stream from HBM in bf16 (8MB/core): xt (d-major) feeds
the e-matmul, xc (t-major) feeds the context matmul - the PE does every
contraction, nothing big runs on the DVE. Energies are computed directly
in t-on-partition form (aT[p, i] = energy(t = i*128+p)) via 8 tiny
matmuls per batch with contiguous [10, 128] slices of e as lhsT, so
exp and the max(exp,1)=exp(relu) fold run at full partition width and
the softmax weights feed the context matmul straight from SBUF (no
broadcast, no DRAM bounce). Softmax denominators come from a
ones-matmul over aT partitions. Output is the unnormalized context +
per-batch denominators; the host divides.
"""

import numpy as np

B, T, D, H = 32, 1024, 512, 10
NCORES = 8
BP = B // NCORES
NT = T // 128
ND = D // 128

_CACHE = {}


def _build_nc():
    from contextlib import ExitStack

    import concourse.bass as bass
    import concourse.mybir as mybir
    import concourse.tile as tile
    from concourse import bacc

    f32 = mybir.dt.float32
    bf16 = mybir.dt.bfloat16
    Alu = mybir.AluOpType
    Act = mybir.ActivationFunctionType

    nc = bacc.Bacc("TRN2", target_bir_lowering=False, debug=False, num_devices=NCORES)

    f8 = mybir.dt.float8e4
    xt_d = nc.dram_tensor("xt", [BP, ND, 128, T], f8, kind="ExternalInput")
    cb8_d = nc.dram_tensor("cb8", [128, 40], f8, kind="ExternalInput")
    xc_d = nc.dram_tensor("xc", [BP, T, D], bf16, kind="ExternalInput")
    cb_d = nc.dram_tensor("cb", [128, 48], bf16, kind="ExternalInput")
    cf_d = nc.dram_tensor("cf", [128, 8], f32, kind="ExternalInput")
    ctx_d = nc.dram_tensor("ctx_out", [1, BP * D], f32, kind="ExternalOutput")
    den_d = nc.dram_tensor("den_out", [1, BP], f32, kind="ExternalOutput")

    with tile.TileContext(nc) as tc, ExitStack() as ctx:
        consts = ctx.enter_context(tc.tile_pool(name="consts", bufs=1))
        xsb = ctx.enter_context(tc.tile_pool(name="xsb", bufs=BP))
        xcb = ctx.enter_context(tc.tile_pool(name="xcb", bufs=BP))
        esb = ctx.enter_context(tc.tile_pool(name="esb", bufs=2))
        asb = ctx.enter_context(tc.tile_pool(name="asb", bufs=2))
        outp = ctx.enter_context(tc.tile_pool(name="outp", bufs=1))
        pe = ctx.enter_context(tc.tile_pool(name="pe", bufs=2, space="PSUM"))
        pa = ctx.enter_context(tc.tile_pool(name="pa", bufs=1, space="PSUM"))
        pdp = ctx.enter_context(tc.tile_pool(name="pdp", bufs=1, space="PSUM"))
        pcx = ctx.enter_context(tc.tile_pool(name="pcx", bufs=2, space="PSUM"))

        # ---- input DMAs: sync ring b0/b1, act ring consts + b2/b3 ----
        cb = consts.tile([128, 48], bf16)
        nc.scalar.dma_start(out=cb, in_=cb_d[:, :])
        cb8 = consts.tile([128, ND, H], f8)
        nc.scalar.dma_start(out=cb8, in_=cb8_d[:, :].rearrange("p (J h) -> p J h", J=ND))
        cf = consts.tile([128, 8], f32)
        nc.scalar.dma_start(out=cf, in_=cf_d[:, :])

        xt_tiles = [None] * BP
        xc_tiles = [None] * BP

        def load_xt(b):
            xt = xsb.tile([128, ND, T], f8, tag="xt", name=f"xt{b}")
            src = xt_d[b].rearrange("J p t -> p J t")
            nc.sync.dma_start(out=xt[:, :, 0:512], in_=src[:, :, 0:512])
            nc.sync.dma_start(out=xt[:, :, 512:1024], in_=src[:, :, 512:1024])
            xt_tiles[b] = xt

        def load_xc(b):
            xc = xcb.tile([128, NT, D], bf16, tag="xc", name=f"xc{b}")
            srcc = xc_d[b].rearrange("(i p) d -> p i d", p=128)
            nc.sync.dma_start(out=xc[:, 0:4, :], in_=srcc[:, 0:4, :])
            nc.sync.dma_start(out=xc[:, 4:8, :], in_=srcc[:, 4:8, :])
            xc_tiles[b] = xc

        # all x on the sync ring; xt's first (they gate each batch's long
        # e->tanh->energies chain), xc's later (they only gate the short
        # final context matmul)
        load_xt(0)
        load_xt(1)
        load_xt(2)
        load_xt(3)
        load_xc(0)
        load_xc(1)
        load_xc(2)
        load_xc(3)

        junk = consts.tile([128, 512], bf16)
        nc.vector.memset(junk, 0.0)
        wz = consts.tile([128, 10], bf16)
        nc.vector.memset(wz, 0.0)
        ones = consts.tile([128, 1], bf16)
        nc.vector.memset(ones, 1.0)
        jrow = consts.tile([1, NT], f32)

        ctxo = outp.tile([1, BP * D], f32)
        dent = outp.tile([1, BP], f32)

        # ---- PE pre-warm while the first x DMA is in flight ----
        for _ in range(2):
            wps = pe.tile([10, 512], f32, tag="peh0", name="warm")
            nc.tensor.matmul(wps, wz, junk, start=True, stop=True)

        e_ps = [None] * BP
        e_t = [None] * BP
        aT_ps = [None] * BP
        aTm = [None] * BP
        cps = [None] * BP

        def e_mm(b):
            # fp8 DoubleRow: one matmul accumulates two J-chunks (2 rows/cycle)
            e_ps[b] = [
                pe.tile([10, 512], f32, tag=f"peh{h}", name=f"eps{b}_{h}")
                for h in range(2)
            ]
            for h in range(2):
                for J in range(ND):
                    nc.tensor.matmul(
                        e_ps[b][h],
                        cb8[:, J, :],
                        xt_tiles[b][:, J, h * 512 : (h + 1) * 512],
                        start=(J == 0),
                        stop=(J == ND - 1),
                    )

        def tanh(b, h):
            if h == 0:
                e_t[b] = esb.tile([10, 1024], bf16, tag="e", name=f"e{b}")
            nc.scalar.activation(
                e_t[b][:, h * 512 : (h + 1) * 512],
                e_ps[b][h],
                Act.Tanh,
                bias=cf[0:10, b : b + 1],
                scale=1.0,
            )

        def en2(b):
            # aT_ps[p, i] = energy(t = i*128+p); lhsT = contiguous e chunk
            aT_ps[b] = pa.tile([128, NT], f32, tag="aT", name=f"aT{b}")
            for i in range(NT):
                nc.tensor.matmul(
                    aT_ps[b][:, i : i + 1],
                    e_t[b][:, i * 128 : (i + 1) * 128],
                    cb[0:10, 40:41],
                    start=True,
                    stop=True,
                )

        def exp_max(b):
            aT_e = asb.tile([128, NT], bf16, tag="aTe", name=f"aTe{b}")
            nc.scalar.activation(aT_e, aT_ps[b], Act.Exp, bias=cf[:, 4:5], scale=1.0)
            aTm[b] = asb.tile([128, NT], bf16, tag="aTm", name=f"aTm{b}")
            nc.vector.tensor_scalar(aTm[b], aT_e, 1.0, None, Alu.max)

        def den(b):
            dps = pdp.tile([1, NT], f32, tag="dps", name=f"dps{b}")
            nc.tensor.matmul(dps, ones, aTm[b], start=True, stop=True)
            nc.vector.tensor_scalar(
                jrow, dps, 1.0, None, Alu.mult, Alu.add,
                accum_out=dent[:, b : b + 1],
            )

        def ctx_mm(b):
            cps[b] = pcx.tile([1, D], f32, tag="cps", name=f"cps{b}")
            for i in range(NT):
                nc.tensor.matmul(
                    cps[b],
                    aTm[b][:, i : i + 1],
                    xc_tiles[b][:, i, :],
                    start=(i == 0),
                    stop=(i == NT - 1),
                )

        def finish(b):
            nc.vector.tensor_scalar(
                ctxo[:, b * D : (b + 1) * D], cps[b], 1.0, None, Alu.mult
            )
            # stream this batch's raw context out immediately (act ring idle)
            nc.scalar.dma_start(
                out=ctx_d[0:1, b * D : (b + 1) * D],
                in_=ctxo[:, b * D : (b + 1) * D],
            )

        def head(b):
            tanh(b, 0)
            tanh(b, 1)
            en2(b)
            exp_max(b)

        # ---- pipeline: batch order matches DMA arrival ----
        S = [0, 1, 2, 3]
        e_mm(S[0])
        head(S[0])
        e_mm(S[1])
        ctx_mm(S[0])
        head(S[1])
        e_mm(S[2])
        finish(S[0])
        ctx_mm(S[1])
        head(S[2])
        e_mm(S[3])
        finish(S[1])
        ctx_mm(S[2])
        head(S[3])
        finish(S[2])
        ctx_mm(S[3])
        finish(S[3])
        for b in S:
            den(b)

        nc.scalar.dma_start(out=den_d[:, :], in_=dent)

    nc.compile()
    return nc


def _get_nc():
    if "nc" not in _CACHE:
        _CACHE["nc"] = _build_nc()
    return _CACHE["nc"]


def _make_in_maps(cbhg, rnn, w1, b1, w2, b2):
    import ml_dtypes

    bf16 = ml_dtypes.bfloat16
    w1b = np.asarray(w1[D:], dtype=np.float64)
    f8 = ml_dtypes.float8_e4m3
    xc_all = np.ascontiguousarray(cbhg.astype(bf16))
    xt_all = np.ascontiguousarray(
        cbhg.astype(f8).reshape(B, T, ND, 128).transpose(0, 2, 3, 1)
    )
    w1a = w1[:D].reshape(ND, 128, H).transpose(1, 0, 2).reshape(128, 40)
    cb = np.zeros((128, 48), dtype=bf16)
    cb[:, 0:40] = w1a
    cb[0:10, 40] = w2[:, 0]
    cb8 = np.ascontiguousarray(w1a.astype(f8))
    maps = []
    for c in range(NCORES):
        rnn_c = np.asarray(rnn[c * BP : (c + 1) * BP], dtype=np.float64)
        rbm = (rnn_c @ w1b + b1.astype(np.float64)).T
        cf = np.zeros((128, 8), dtype=np.float32)
        cf[0:10, 0:BP] = rbm
        cf[:, 4] = b2[0]
        maps.append(
            {
                "xt": np.ascontiguousarray(xt_all[c * BP : (c + 1) * BP]),
                "xc": np.ascontiguousarray(xc_all[c * BP : (c + 1) * BP]),
                "cb": cb,
                "cb8": cb8,
                "cf": cf,
            }
        )
    return maps


def _unpack_out(ctx_raw, den_raw):
    """[1, BP*D] raw context + [1, BP] denominators -> [BP, D] f32."""
    ctx = ctx_raw.reshape(BP, D)
    return (ctx.astype(np.float64) / den_raw.reshape(BP, 1).astype(np.float64)).astype(
        np.float32
    )


def _run(in_maps, trace=False):
    from concourse.bass_utils import run_bass_kernel_spmd

    nc = _get_nc()
    return run_bass_kernel_spmd(nc, in_maps, core_ids=list(range(NCORES)), trace=trace)


def kernel(cbhg_encoding, attention_rnn_output, W1, b1, W2, b2):
    cbhg = np.asarray(cbhg_encoding, dtype=np.float32)
    rnn = np.asarray(attention_rnn_output, dtype=np.float32)
    w1 = np.ascontiguousarray(np.asarray(W1, dtype=np.float32))
    b1v = np.ascontiguousarray(np.asarray(b1, dtype=np.float32))
    w2 = np.ascontiguousarray(np.asarray(W2, dtype=np.float32))
    b2v = np.ascontiguousarray(np.asarray(b2, dtype=np.float32))

    res = _run(_make_in_maps(cbhg, rnn, w1, b1v, w2, b2v))
    context = np.concatenate(
        [
            _unpack_out(res.results[c]["ctx_out"], res.results[c]["den_out"])[:, None, :]
            for c in range(NCORES)
        ],
        axis=0,
    ).astype(np.float32)
    rnn_reshaped = rnn.reshape(B, 1, D).copy()
    return (context, rnn_reshaped)


# revision 31
# speedup vs baseline: 1.0069x; 1.0069x over previous
"""Trainium2 Bass kernel v7: dual-layout x load (fp8 + bf16), all-PE datapath.

Two x layouts stream from HBM on one deep-queued sync HWDGE ring
(6MB/core): xt (d-major, fp8_e4m3) feeds the e-matmul, xc (t-major,
bf16) feeds the context matmul - the PE does every contraction, nothing
big runs on the DVE. fp8 on the energies path costs ~9.3e-3 rel err
(gate 2e-2); fp8 on the context path would cost 1.8e-2 and is avoided.
Energies are computed directly
in t-on-partition form (aT[p, i] = energy(t = i*128+p)) via 8 tiny
matmuls per batch with contiguous [10, 128] slices of e as lhsT, so
exp and the max(exp,1)=exp(relu) fold run at full partition width and
the softmax weights feed the context matmul straight from SBUF (no
broadcast, no DRAM bounce). Softmax denominators come from a
ones-matmul over aT partitions. Output is the unnormalized context +
per-batch denominators; the host divides.
"""

import numpy as np

B, T, D, H = 32, 1024, 512, 10
NCORES = 8
BP = B // NCORES
NT = T // 128
ND = D // 128

_CACHE = {}


def _build_nc():
    from contextlib import ExitStack

    import concourse.bass as bass
    import concourse.mybir as mybir
    import concourse.tile as tile
    from concourse import bacc

    f32 = mybir.dt.float32
    bf16 = mybir.dt.bfloat16
    Alu = mybir.AluOpType
    Act = mybir.ActivationFunctionType

    nc = bacc.Bacc("TRN2", target_bir_lowering=False, debug=False, num_devices=NCORES)

    f8 = mybir.dt.float8e4
    xt_d = nc.dram_tensor("xt", [BP, ND, 128, T], f8, kind="ExternalInput")
    cb8_d = nc.dram_tensor("cb8", [128, 40], f8, kind="ExternalInput")
    xc_d = nc.dram_tensor("xc", [BP, T, D], bf16, kind="ExternalInput")
    cb_d = nc.dram_tensor("cb", [128, 48], bf16, kind="ExternalInput")
    cf_d = nc.dram_tensor("cf", [128, 8], f32, kind="ExternalInput")
    ctx_d = nc.dram_tensor("ctx_out", [1, BP * D], f32, kind="ExternalOutput")
    den_d = nc.dram_tensor("den_out", [1, BP], f32, kind="ExternalOutput")

    with tile.TileContext(nc) as tc, ExitStack() as ctx:
        consts = ctx.enter_context(tc.tile_pool(name="consts", bufs=1))
        xsb = ctx.enter_context(tc.tile_pool(name="xsb", bufs=BP))
        xcb = ctx.enter_context(tc.tile_pool(name="xcb", bufs=BP))
        esb = ctx.enter_context(tc.tile_pool(name="esb", bufs=2))
        asb = ctx.enter_context(tc.tile_pool(name="asb", bufs=2))
        outp = ctx.enter_context(tc.tile_pool(name="outp", bufs=1))
        pe = ctx.enter_context(tc.tile_pool(name="pe", bufs=2, space="PSUM"))
        pa = ctx.enter_context(tc.tile_pool(name="pa", bufs=1, space="PSUM"))
        pdp = ctx.enter_context(tc.tile_pool(name="pdp", bufs=1, space="PSUM"))
        pcx = ctx.enter_context(tc.tile_pool(name="pcx", bufs=2, space="PSUM"))

        # ---- input DMAs: sync ring b0/b1, act ring consts + b2/b3 ----
        cb = consts.tile([128, 48], bf16)
        nc.scalar.dma_start(out=cb, in_=cb_d[:, :])
        cb8 = consts.tile([128, ND, H], f8)
        nc.scalar.dma_start(out=cb8, in_=cb8_d[:, :].rearrange("p (J h) -> p J h", J=ND))
        cf = consts.tile([128, 8], f32)
        nc.scalar.dma_start(out=cf, in_=cf_d[:, :])

        xt_tiles = [None] * BP
        xc_tiles = [None] * BP

        def load_xt(b):
            xt = xsb.tile([128, ND, T], f8, tag="xt", name=f"xt{b}")
            src = xt_d[b].rearrange("J p t -> p J t")
            nc.sync.dma_start(out=xt[:, :, 0:512], in_=src[:, :, 0:512])
            nc.sync.dma_start(out=xt[:, :, 512:1024], in_=src[:, :, 512:1024])
            xt_tiles[b] = xt

        def load_xc(b):
            xc = xcb.tile([128, NT, D], bf16, tag="xc", name=f"xc{b}")
            srcc = xc_d[b].rearrange("(i p) d -> p i d", p=128)
            nc.sync.dma_start(out=xc[:, 0:4, :], in_=srcc[:, 0:4, :])
            nc.sync.dma_start(out=xc[:, 4:8, :], in_=srcc[:, 4:8, :])
            xc_tiles[b] = xc

        # all x on the sync ring; xt's first (they gate each batch's long
        # e->tanh->energies chain), xc's later (they only gate the short
        # final context matmul)
        load_xt(0)
        load_xt(1)
        load_xt(2)
        load_xt(3)
        load_xc(0)
        load_xc(1)
        load_xc(2)
        load_xc(3)

        junk = consts.tile([128, 512], bf16)
        nc.vector.memset(junk, 0.0)
        wz = consts.tile([128, 10], bf16)
        nc.vector.memset(wz, 0.0)
        ones = consts.tile([128, 1], bf16)
        nc.vector.memset(ones, 1.0)
        jrow = consts.tile([1, NT], f32)

        ctxo = outp.tile([1, BP * D], f32)
        dent = outp.tile([1, BP], f32)

        # ---- PE pre-warm while the first x DMA is in flight ----
        for _ in range(2):
            wps = pe.tile([10, 512], f32, tag="peh0", name="warm")
            nc.tensor.matmul(wps, wz, junk, start=True, stop=True)

        e_ps = [None] * BP
        e_t = [None] * BP
        aT_ps = [None] * BP
        aTm = [None] * BP
        cps = [None] * BP

        def e_mm(b):
            # fp8 DoubleRow: one matmul accumulates two J-chunks (2 rows/cycle)
            e_ps[b] = [
                pe.tile([10, 512], f32, tag=f"peh{h}", name=f"eps{b}_{h}")
                for h in range(2)
            ]
            for h in range(2):
                for J in range(ND):
                    nc.tensor.matmul(
                        e_ps[b][h],
                        cb8[:, J, :],
                        xt_tiles[b][:, J, h * 512 : (h + 1) * 512],
                        start=(J == 0),
                        stop=(J == ND - 1),
                    )

        def tanh(b, h):
            if h == 0:
                e_t[b] = esb.tile([10, 1024], bf16, tag="e", name=f"e{b}")
            nc.scalar.activation(
                e_t[b][:, h * 512 : (h + 1) * 512],
                e_ps[b][h],
                Act.Tanh,
                bias=cf[0:10, b : b + 1],
                scale=1.0,
            )

        def en2(b):
            # aT_ps[p, i] = energy(t = i*128+p); lhsT = contiguous e chunk
            aT_ps[b] = pa.tile([128, NT], f32, tag="aT", name=f"aT{b}")
            for i in range(NT):
                nc.tensor.matmul(
                    aT_ps[b][:, i : i + 1],
                    e_t[b][:, i * 128 : (i + 1) * 128],
                    cb[0:10, 40:41],
                    start=True,
                    stop=True,
                )

        def exp_max(b):
            aT_e = asb.tile([128, NT], bf16, tag="aTe", name=f"aTe{b}")
            nc.scalar.activation(aT_e, aT_ps[b], Act.Exp, bias=cf[:, 4:5], scale=1.0)
            aTm[b] = asb.tile([128, NT], bf16, tag="aTm", name=f"aTm{b}")
            nc.vector.tensor_scalar(aTm[b], aT_e, 1.0, None, Alu.max)

        def den(b):
            dps = pdp.tile([1, NT], f32, tag="dps", name=f"dps{b}")
            nc.tensor.matmul(dps, ones, aTm[b], start=True, stop=True)
            nc.vector.tensor_scalar(
                jrow, dps, 1.0, None, Alu.mult, Alu.add,
                accum_out=dent[:, b : b + 1],
            )

        def ctx_mm(b):
            cps[b] = pcx.tile([1, D], f32, tag="cps", name=f"cps{b}")
            for i in range(NT):
                nc.tensor.matmul(
                    cps[b],
                    aTm[b][:, i : i + 1],
                    xc_tiles[b][:, i, :],
                    start=(i == 0),
                    stop=(i == NT - 1),
                )

        def finish(b):
            nc.vector.tensor_scalar(
                ctxo[:, b * D : (b + 1) * D], cps[b], 1.0, None, Alu.mult
            )
            # stream this batch's raw context out immediately (act ring idle)
            nc.scalar.dma_start(
                out=ctx_d[0:1, b * D : (b + 1) * D],
                in_=ctxo[:, b * D : (b + 1) * D],
            )

        def head(b):
            tanh(b, 0)
            tanh(b, 1)
            en2(b)
            exp_max(b)

        # ---- pipeline: batch order matches DMA arrival ----
        S = [0, 1, 2, 3]
        e_mm(S[0])
        head(S[0])
        e_mm(S[1])
        ctx_mm(S[0])
        head(S[1])
        e_mm(S[2])
        finish(S[0])
        ctx_mm(S[1])
        head(S[2])
        e_mm(S[3])
        finish(S[1])
        ctx_mm(S[2])
        head(S[3])
        finish(S[2])
        ctx_mm(S[3])
        finish(S[3])
        for b in S:
            den(b)

        nc.scalar.dma_start(out=den_d[:, :], in_=dent)

    nc.compile()
    return nc


def _get_nc():
    if "nc" not in _CACHE:
        _CACHE["nc"] = _build_nc()
    return _CACHE["nc"]


def _make_in_maps(cbhg, rnn, w1, b1, w2, b2):
    import ml_dtypes

    bf16 = ml_dtypes.bfloat16
    w1b = np.asarray(w1[D:], dtype=np.float64)
    f8 = ml_dtypes.float8_e4m3
    xc_all = np.ascontiguousarray(cbhg.astype(bf16))
    xt_all = np.ascontiguousarray(
        cbhg.astype(f8).reshape(B, T, ND, 128).transpose(0, 2, 3, 1)
    )
    w1a = w1[:D].reshape(ND, 128, H).transpose(1, 0, 2).reshape(128, 40)
    cb = np.zeros((128, 48), dtype=bf16)
    cb[:, 0:40] = w1a
    cb[0:10, 40] = w2[:, 0]
    cb8 = np.ascontiguousarray(w1a.astype(f8))
    maps = []
    for c in range(NCORES):
        rnn_c = np.asarray(rnn[c * BP : (c + 1) * BP], dtype=np.float64)
        rbm = (rnn_c @ w1b + b1.astype(np.float64)).T
        cf = np.zeros((128, 8), dtype=np.float32)
        cf[0:10, 0:BP] = rbm
        cf[:, 4] = b2[0]
        maps.append(
            {
                "xt": np.ascontiguousarray(xt_all[c * BP : (c + 1) * BP]),
                "xc": np.ascontiguousarray(xc_all[c * BP : (c + 1) * BP]),
                "cb": cb,
                "cb8": cb8,
                "cf": cf,
            }
        )
    return maps


def _unpack_out(ctx_raw, den_raw):
    """[1, BP*D] raw context + [1, BP] denominators -> [BP, D] f32."""
    ctx = ctx_raw.reshape(BP, D)
    return (ctx.astype(np.float64) / den_raw.reshape(BP, 1).astype(np.float64)).astype(
        np.float32
    )


def _run(in_maps, trace=False):
    from concourse.bass_utils import run_bass_kernel_spmd

    nc = _get_nc()
    return run_bass_kernel_spmd(nc, in_maps, core_ids=list(range(NCORES)), trace=trace)


def kernel(cbhg_encoding, attention_rnn_output, W1, b1, W2, b2):
    cbhg = np.asarray(cbhg_encoding, dtype=np.float32)
    rnn = np.asarray(attention_rnn_output, dtype=np.float32)
    w1 = np.ascontiguousarray(np.asarray(W1, dtype=np.float32))
    b1v = np.ascontiguousarray(np.asarray(b1, dtype=np.float32))
    w2 = np.ascontiguousarray(np.asarray(W2, dtype=np.float32))
    b2v = np.ascontiguousarray(np.asarray(b2, dtype=np.float32))

    res = _run(_make_in_maps(cbhg, rnn, w1, b1v, w2, b2v))
    context = np.concatenate(
        [
            _unpack_out(res.results[c]["ctx_out"], res.results[c]["den_out"])[:, None, :]
            for c in range(NCORES)
        ],
        axis=0,
    ).astype(np.float32)
    rnn_reshaped = rnn.reshape(B, 1, D).copy()
    return (context, rnn_reshaped)
